# revision 4
# baseline (speedup 1.0000x reference)
"""ChebyKAN layer kernel for Trainium2 (8 NeuronCores).

Computes y[b,o] = sum_{i,d} T_d(tanh(x)[b,i]) * C[i,o,d] for
x: (8192, 1024) f32, C: (1024, 1024, 9) f32, i.e. a (8192 x 8192) @
(8192 x 1024) matmul after building product-basis features.

Sharding: 4-way over batch rows x 2-way over output columns
(core c -> batch group c//2, output group c%2). No collectives.

On-chip math: the Chebyshev basis is replaced by a product-feature basis
that needs only 1 multiply (+ occasional affine) per degree:
    F1 = t            = T1
    F2 = t*t          = (T2+1)/2        t2 = 2*F2-1 = T2
    F3 = t*t2         = (T3+T1)/2
    F4 = t2*t2        = (T4+1)/2        t4 = 2*F4-1 = T4
    F5 = t*t4         = (T5+T3)/2
    F6 = t2*t4        = (T6+T2)/2
    F7 = t4*F3        = (T7+T5+T3+T1)/4
    F8 = t4*t4        = (T8+1)/2
F is a triangular, well-conditioned linear transform of T, so the
weights are re-expressed host-side: y = bias + sum_k F_k V_k with
    V0 = W0 - W2 - W4 + W6 - W8   (constant feature -> host-side bias)
    V1 = W1 - W3 + W5 - W7
    V2 = 2(W2 - W6);  V3 = 2(W3 - W5);  V4 = 2 W4
    V5 = 2(W5 - W7);  V6 = 2 W6;  V7 = 4 W7;  V8 = 2 W8
The bias term sum_i V0[i,o] is added on the host after the gather.

Everything on-chip runs in fp16: the feature chain tiles ARE the matmul
stationary operands (no bf16 conversion copies), and the weights are
host-scaled by 2^9 into fp16 range (undone exactly in the psum-evac
copy). Matmuls are emitted as 64-long same-PSUM-bank accumulation
chains: back-to-back matmuls into one bank hide the LD_WEIGHTS
(stationary load) entirely (~175 ns / 512-col matmul measured, vs
~256 ns when alternating banks).
"""

from contextlib import ExitStack

import ml_dtypes
import numpy as np

import concourse.bacc as bacc
import concourse.mybir as mybir
import concourse.tile as tile
from concourse.bass_utils import run_bass_kernel_spmd

P = 128
B_FULL, I_DIM, O_FULL, DEG = 8192, 1024, 1024, 8
N_CORES = 8
BG, OG = 4, 2  # core grid: batch groups x output groups
B_SH = B_FULL // BG  # 2048 batch rows per core
O_SH = O_FULL // OG  # 512 output cols per core
CH = 256  # batch-chunk width per group
NG = B_SH // CH  # 8 groups
NBT = CH // P  # 2 psum bank-chains per group
NIT = I_DIM // P  # 8 i-tiles
KT = DEG * NIT  # 64 contraction tiles
WSCALE = 512.0  # host weight scale into fp16 range (undone in psum evac)
F32 = mybir.dt.float32
F16 = mybir.dt.float16
MULT = mybir.AluOpType.mult
ADD = mybir.AluOpType.add
TANH = mybir.ActivationFunctionType.Tanh
COPY = mybir.ActivationFunctionType.Copy

_NC_CACHE = []


def _build_ir(loop_iters=None, variant="full"):
    nc = bacc.Bacc(
        "TRN2", target_bir_lowering=False, debug=False, enable_asserts=False
    )
    xT = nc.dram_tensor("xT", [I_DIM, B_SH], F32, kind="ExternalInput").ap()
    wv = nc.dram_tensor("wv", [KT * P, O_SH], F16, kind="ExternalInput").ap()
    y = nc.dram_tensor("y", [B_SH, O_SH], F32, kind="ExternalOutput").ap()

    do_mm = variant in ("full", "pe")
    do_prod = variant in ("full", "prod")

    with ExitStack() as ctx:
        tc = ctx.enter_context(tile.TileContext(nc))
        wpool = ctx.enter_context(tc.tile_pool(name="w", bufs=1))
        fpool = ctx.enter_context(tc.tile_pool(name="f", bufs=2))
        cpool = ctx.enter_context(tc.tile_pool(name="c", bufs=2))
        xpool = ctx.enter_context(tc.tile_pool(name="x", bufs=3))
        ypool = ctx.enter_context(tc.tile_pool(name="yp", bufs=4))
        pspool = ctx.enter_context(tc.tile_pool(name="ps", bufs=4, space="PSUM"))

        wt = [wpool.tile([P, O_SH], F16, tag=f"w{k}", name=f"w{k}") for k in range(KT)]
        for k in range(KT):
            nc.sync.dma_start(out=wt[k][:], in_=wv[k * P : (k + 1) * P, :])

        fixed_feat = {}
        if variant == "pe":
            fpool_pe = ctx.enter_context(tc.tile_pool(name="fpe", bufs=1))
            for it in range(NIT):
                for d in range(1, DEG + 1):
                    t = fpool_pe.tile([P, CH], F16, tag=f"pf{d}_{it}",
                                      name=f"pf{d}_{it}")
                    nc.vector.memset(t[:], 0.01 * d)
                    fixed_feat[(d, it)] = t

        def emit_body():
          for g in range(NG):
            b0 = g * CH
            feats = {}
            if do_prod:
                for it in range(NIT):
                    xr = xpool.tile([P, CH], F32, tag="xr", name="xr")
                    nc.sync.dma_start(
                        out=xr[:], in_=xT[it * P : (it + 1) * P, b0 : b0 + CH]
                    )
                    f = {}
                    for d in range(1, DEG + 1):
                        f[d] = fpool.tile([P, CH], F16, tag=f"f{d}_{it}",
                                          name=f"f{d}_{it}")
                    t2 = cpool.tile([P, CH], F16, tag=f"t2_{it}", name=f"t2_{it}")
                    t4 = cpool.tile([P, CH], F16, tag=f"t4_{it}", name=f"t4_{it}")
                    nc.scalar.activation(f[1][:], xr[:], TANH)
                    nc.vector.tensor_tensor(f[2][:], f[1][:], f[1][:], MULT)
                    nc.vector.tensor_scalar(t2[:], f[2][:], 2.0, -1.0, MULT, ADD)
                    nc.vector.tensor_tensor(f[3][:], f[1][:], t2[:], MULT)
                    nc.vector.tensor_tensor(f[4][:], t2[:], t2[:], MULT)
                    nc.vector.tensor_scalar(t4[:], f[4][:], 2.0, -1.0, MULT, ADD)
                    nc.vector.tensor_tensor(f[5][:], f[1][:], t4[:], MULT)
                    nc.vector.tensor_tensor(f[6][:], t2[:], t4[:], MULT)
                    nc.vector.tensor_tensor(f[7][:], t4[:], f[3][:], MULT)
                    nc.vector.tensor_tensor(f[8][:], t4[:], t4[:], MULT)
                    for d in range(1, DEG + 1):
                        feats[(d, it)] = f[d]
            else:
                feats = fixed_feat

            if do_mm:
                psums = [
                    pspool.tile([P, O_SH], F32, tag=f"ps{bt}", name=f"ps{bt}")
                    for bt in range(NBT)
                ]
                for bt in range(NBT):
                    for it in range(NIT):
                        for d in range(1, DEG + 1):
                            k = it * DEG + (d - 1)
                            nc.tensor.matmul(
                                psums[bt][:],
                                feats[(d, it)][:, bt * P : (bt + 1) * P],
                                wt[k][:],
                                start=(k == 0),
                                stop=(k == KT - 1),
                            )
                for bt in range(NBT):
                    ysb = ypool.tile([P, O_SH], F32, tag="ysb", name="ysb")
                    nc.scalar.activation(
                        ysb[:], psums[bt][:], COPY, scale=1.0 / WSCALE
                    )
                    nc.sync.dma_start(
                        out=y[b0 + bt * P : b0 + (bt + 1) * P, :], in_=ysb[:]
                    )
            else:
                ysb = ypool.tile([P, O_SH], F32, tag="ysb", name="ysb")
                nc.scalar.copy(ysb[:, 0:CH], feats[(8, 0)][:])
                nc.sync.dma_start(out=y[b0 : b0 + P, 0:CH], in_=ysb[:, 0:CH])

        if loop_iters is not None:
            with tc.For_i(0, loop_iters, 1):
                emit_body()
        else:
            emit_body()
    nc.compile()
    return nc


def get_nc():
    if not _NC_CACHE:
        _NC_CACHE.append(_build_ir())
    return _NC_CACHE[0]


def _v_basis(cheby_coeffs):
    c = np.asarray(cheby_coeffs, dtype=np.float64)
    w = [c[:, :, d] for d in range(DEG + 1)]
    return [
        w[0] - w[2] - w[4] + w[6] - w[8],
        w[1] - w[3] + w[5] - w[7],
        2.0 * (w[2] - w[6]),
        2.0 * (w[3] - w[5]),
        2.0 * w[4],
        2.0 * (w[5] - w[7]),
        2.0 * w[6],
        4.0 * w[7],
        2.0 * w[8],
    ]


def prep_inputs(x, cheby_coeffs):
    """Host-side shard prep: returns per-core input maps."""
    x = np.asarray(x, dtype=np.float32)
    v = _v_basis(cheby_coeffs)
    # weight layout: k = it*8 + (d-1)  ->  rows of V_d for i-tile `it`
    wv_full = np.concatenate(
        [v[d][it * P : (it + 1) * P, :] for it in range(NIT)
         for d in range(1, DEG + 1)],
        axis=0,
    )  # (8192, 1024)
    wv_f16 = (wv_full * WSCALE).astype(np.float16)
    xt_full = np.ascontiguousarray(x.T)  # (1024, 8192)

    in_maps = []
    for core in range(N_CORES):
        bg, og = core // OG, core % OG
        in_maps.append(
            {
                "xT": np.ascontiguousarray(
                    xt_full[:, bg * B_SH : (bg + 1) * B_SH]
                ),
                "wv": np.ascontiguousarray(
                    wv_f16[:, og * O_SH : (og + 1) * O_SH]
                ),
            }
        )
    return in_maps


def assemble_output(results, bias):
    y_full = np.empty((B_FULL, O_FULL), dtype=np.float32)
    for core in range(N_CORES):
        bg, og = core // OG, core % OG
        y_full[bg * B_SH : (bg + 1) * B_SH, og * O_SH : (og + 1) * O_SH] = (
            np.asarray(results[core]["y"], dtype=np.float32)
        )
    y_full += bias[None, :].astype(np.float32)
    return y_full


def kernel(x, cheby_coeffs):
    nc = get_nc()
    in_maps = prep_inputs(x, cheby_coeffs)
    bias = _v_basis(cheby_coeffs)[0].sum(axis=0)
    res = run_bass_kernel_spmd(nc, in_maps, list(range(N_CORES)))
    return assemble_output(res.results, bias)


# revision 9
# speedup vs baseline: 1.1760x; 1.1760x over previous
"""ChebyKAN layer kernel for Trainium2 (8 NeuronCores).

Computes y[b,o] = sum_{i,d} T_d(tanh(x)[b,i]) * C[i,o,d] for
x: (8192, 1024) f32, C: (1024, 1024, 9) f32, i.e. a (8192 x 8192) @
(8192 x 1024) matmul after building product-basis features.

Sharding: 4-way over batch rows x 2-way over output columns
(core c -> batch group c//2, output group c%2). No collectives.

On-chip math: the Chebyshev basis is replaced by a product-feature basis
that needs only 1 multiply (+ occasional affine) per degree:
    F1 = t            = T1
    F2 = t*t          = (T2+1)/2        t2 = 2*F2-1 = T2
    F3 = t*t2         = (T3+T1)/2
    F4 = t2*t2        = (T4+1)/2        t4 = 2*F4-1 = T4
    F5 = t*t4         = (T5+T3)/2
    F6 = t2*t4        = (T6+T2)/2
    F7 = t4*F3        = (T7+T5+T3+T1)/4
    F8 = t4*t4        = (T8+1)/2
F is a triangular, well-conditioned linear transform of T, so the
weights are re-expressed host-side: y = bias + sum_k F_k V_k with
    V0 = W0 - W2 - W4 + W6 - W8   (constant feature -> host-side bias)
    V1 = W1 - W3 + W5 - W7
    V2 = 2(W2 - W6);  V3 = 2(W3 - W5);  V4 = 2 W4
    V5 = 2(W5 - W7);  V6 = 2 W6;  V7 = 4 W7;  V8 = 2 W8
The bias term sum_i V0[i,o] is added on the host after the gather.

The feature chain runs in fp16 on DVE/Act; matmul operands are bf16
(f16 shows a data-dependent PE slow path on random-mantissa moving
data — up to 1.7x — while bf16 streams at full rate; see probe.py).
Matmuls are emitted as 64-long same-PSUM-bank accumulation chains:
back-to-back matmuls into one bank hide the LD_WEIGHTS (stationary
load) entirely (~175-220 ns / 512-col matmul vs ~256 ns when
alternating banks; the shared pool's PE clock drifts ~1.5x between
sessions, so absolute ns vary).
"""

from contextlib import ExitStack

import ml_dtypes
import numpy as np

import concourse.bacc as bacc
import concourse.mybir as mybir
import concourse.tile as tile
from concourse.bass_utils import run_bass_kernel_spmd

P = 128
B_FULL, I_DIM, O_FULL, DEG = 8192, 1024, 1024, 8
N_CORES = 8
BG, OG = 4, 2  # core grid: batch groups x output groups
B_SH = B_FULL // BG  # 2048 batch rows per core
O_SH = O_FULL // OG  # 512 output cols per core
CH = 256  # batch-chunk width per group
NG = B_SH // CH  # 8 groups
NBT = CH // P  # 2 psum bank-chains per group
NIT = I_DIM // P  # 8 i-tiles
KT = DEG * NIT  # 64 contraction tiles
WSCALE = 512.0  # host weight scale into fp16 range (undone in psum evac)
MM_SCHEME = "bf16"  # "f16" | "bf16" | "mixed" — keep in sync between
                   # _build_ir default and prep_inputs weight dtype
BF16 = mybir.dt.bfloat16
F32 = mybir.dt.float32
F16 = mybir.dt.float16
MULT = mybir.AluOpType.mult
ADD = mybir.AluOpType.add
TANH = mybir.ActivationFunctionType.Tanh
COPY = mybir.ActivationFunctionType.Copy

_NC_CACHE = []


def _build_ir(loop_iters=None, variant="full", fbufs=2, mm=None):
    """mm: matmul operand scheme — "f16" (weights fp16 scaled by WSCALE),
    "bf16" (weights bf16, features converted to bf16 via copies), or
    "mixed" (f16 feature stationary x bf16 weight moving, no copies)."""
    if mm is None:
        mm = MM_SCHEME
    nc = bacc.Bacc(
        "TRN2", target_bir_lowering=False, debug=False, enable_asserts=False
    )
    wdt = F16 if mm == "f16" else BF16
    fdt = BF16 if mm == "bf16" else F16
    xT = nc.dram_tensor("xT", [I_DIM, B_SH], F32, kind="ExternalInput").ap()
    wv = nc.dram_tensor("wv", [KT * P, O_SH], wdt, kind="ExternalInput").ap()
    y = nc.dram_tensor("y", [B_SH, O_SH], F32, kind="ExternalOutput").ap()

    do_mm = variant in ("full", "pe")
    do_prod = variant in ("full", "prod")

    with ExitStack() as ctx:
        tc = ctx.enter_context(tile.TileContext(nc))
        wpool = ctx.enter_context(tc.tile_pool(name="w", bufs=1))
        fpool = ctx.enter_context(tc.tile_pool(name="f", bufs=fbufs))
        cpool = ctx.enter_context(tc.tile_pool(name="c", bufs=2))
        xpool = ctx.enter_context(tc.tile_pool(name="x", bufs=3))
        ypool = ctx.enter_context(tc.tile_pool(name="yp", bufs=4))
        pspool = ctx.enter_context(tc.tile_pool(name="ps", bufs=4, space="PSUM"))

        wt = [wpool.tile([P, O_SH], wdt, tag=f"w{k}", name=f"w{k}") for k in range(KT)]
        for k in range(KT):
            nc.sync.dma_start(out=wt[k][:], in_=wv[k * P : (k + 1) * P, :])

        fixed_feat = {}
        if variant == "pe":
            fpool_pe = ctx.enter_context(tc.tile_pool(name="fpe", bufs=1))
            for it in range(NIT):
                for d in range(1, DEG + 1):
                    t = fpool_pe.tile([P, CH], fdt, tag=f"pf{d}_{it}",
                                      name=f"pf{d}_{it}")
                    nc.vector.memset(t[:], 0.01 * d)
                    fixed_feat[(d, it)] = t

        def emit_body():
          for g in range(NG):
            b0 = g * CH
            feats = {}
            if do_prod:
                for it in range(NIT):
                    xr = xpool.tile([P, CH], F32, tag="xr", name="xr")
                    nc.sync.dma_start(
                        out=xr[:], in_=xT[it * P : (it + 1) * P, b0 : b0 + CH]
                    )
                    f = {}
                    for d in range(1, DEG + 1):
                        f[d] = fpool.tile([P, CH], fdt, tag=f"f{d}_{it}",
                                          name=f"f{d}_{it}")
                    t2 = cpool.tile([P, CH], F16, tag=f"t2_{it}", name=f"t2_{it}")
                    t4 = cpool.tile([P, CH], F16, tag=f"t4_{it}", name=f"t4_{it}")
                    if mm == "bf16":
                        t1 = cpool.tile([P, CH], F16, tag=f"t1_{it}", name=f"t1_{it}")
                        sq1 = cpool.tile([P, CH], F16, tag=f"sq1_{it}", name=f"sq1_{it}")
                        p3 = cpool.tile([P, CH], F16, tag=f"p3_{it}", name=f"p3_{it}")
                        sq2 = cpool.tile([P, CH], F16, tag=f"sq2_{it}", name=f"sq2_{it}")
                        # f16 chain feeding bf16 feature tiles; f1..f4 need
                        # conversion copies, f5..f8 convert in the mul itself
                        nc.scalar.activation(t1[:], xr[:], TANH)
                        nc.vector.tensor_tensor(sq1[:], t1[:], t1[:], MULT)
                        nc.vector.tensor_scalar(t2[:], sq1[:], 2.0, -1.0, MULT, ADD)
                        nc.vector.tensor_tensor(p3[:], t1[:], t2[:], MULT)
                        nc.vector.tensor_tensor(sq2[:], t2[:], t2[:], MULT)
                        nc.vector.tensor_scalar(t4[:], sq2[:], 2.0, -1.0, MULT, ADD)
                        nc.scalar.copy(f[1][:], t1[:])
                        nc.scalar.copy(f[2][:], sq1[:])
                        nc.vector.tensor_copy(f[3][:], p3[:])
                        nc.vector.tensor_copy(f[4][:], sq2[:])
                        nc.vector.tensor_tensor(f[5][:], t1[:], t4[:], MULT)
                        nc.vector.tensor_tensor(f[6][:], t2[:], t4[:], MULT)
                        nc.vector.tensor_tensor(f[7][:], t4[:], p3[:], MULT)
                        nc.vector.tensor_tensor(f[8][:], t4[:], t4[:], MULT)
                    else:
                        nc.scalar.activation(f[1][:], xr[:], TANH)
                        nc.vector.tensor_tensor(f[2][:], f[1][:], f[1][:], MULT)
                        nc.vector.tensor_scalar(t2[:], f[2][:], 2.0, -1.0, MULT, ADD)
                        nc.vector.tensor_tensor(f[3][:], f[1][:], t2[:], MULT)
                        nc.vector.tensor_tensor(f[4][:], t2[:], t2[:], MULT)
                        nc.vector.tensor_scalar(t4[:], f[4][:], 2.0, -1.0, MULT, ADD)
                        nc.vector.tensor_tensor(f[5][:], f[1][:], t4[:], MULT)
                        nc.vector.tensor_tensor(f[6][:], t2[:], t4[:], MULT)
                        nc.vector.tensor_tensor(f[7][:], t4[:], f[3][:], MULT)
                        nc.vector.tensor_tensor(f[8][:], t4[:], t4[:], MULT)
                    for d in range(1, DEG + 1):
                        feats[(d, it)] = f[d]
            else:
                feats = fixed_feat

            if do_mm:
                psums = [
                    pspool.tile([P, O_SH], F32, tag=f"ps{bt}", name=f"ps{bt}")
                    for bt in range(NBT)
                ]
                for bt in range(NBT):
                    for it in range(NIT):
                        for d in range(1, DEG + 1):
                            k = it * DEG + (d - 1)
                            nc.tensor.matmul(
                                psums[bt][:],
                                feats[(d, it)][:, bt * P : (bt + 1) * P],
                                wt[k][:],
                                start=(k == 0),
                                stop=(k == KT - 1),
                            )
                for bt in range(NBT):
                    ysb = ypool.tile([P, O_SH], F32, tag="ysb", name="ysb")
                    scale = (1.0 / WSCALE) if mm == "f16" else 1.0
                    nc.scalar.activation(
                        ysb[:], psums[bt][:], COPY, scale=scale
                    )
                    nc.sync.dma_start(
                        out=y[b0 + bt * P : b0 + (bt + 1) * P, :], in_=ysb[:]
                    )
            else:
                ysb = ypool.tile([P, O_SH], F32, tag="ysb", name="ysb")
                nc.scalar.copy(ysb[:, 0:CH], feats[(8, 0)][:])
                nc.sync.dma_start(out=y[b0 : b0 + P, 0:CH], in_=ysb[:, 0:CH])

        if loop_iters is not None:
            with tc.For_i(0, loop_iters, 1):
                emit_body()
        else:
            emit_body()
    nc.compile()
    return nc


def get_nc():
    if not _NC_CACHE:
        _NC_CACHE.append(_build_ir(mm=MM_SCHEME))
    return _NC_CACHE[0]


def _v_basis(cheby_coeffs):
    c = np.asarray(cheby_coeffs, dtype=np.float64)
    w = [c[:, :, d] for d in range(DEG + 1)]
    return [
        w[0] - w[2] - w[4] + w[6] - w[8],
        w[1] - w[3] + w[5] - w[7],
        2.0 * (w[2] - w[6]),
        2.0 * (w[3] - w[5]),
        2.0 * w[4],
        2.0 * (w[5] - w[7]),
        2.0 * w[6],
        4.0 * w[7],
        2.0 * w[8],
    ]


def prep_inputs(x, cheby_coeffs):
    """Host-side shard prep: returns per-core input maps."""
    x = np.asarray(x, dtype=np.float32)
    v = _v_basis(cheby_coeffs)
    # weight layout: k = it*8 + (d-1)  ->  rows of V_d for i-tile `it`
    wv_full = np.concatenate(
        [v[d][it * P : (it + 1) * P, :] for it in range(NIT)
         for d in range(1, DEG + 1)],
        axis=0,
    )  # (8192, 1024)
    if MM_SCHEME == "f16":
        wv_f16 = (wv_full * WSCALE).astype(np.float16)
    else:
        wv_f16 = wv_full.astype(ml_dtypes.bfloat16)
    xt_full = np.ascontiguousarray(x.T)  # (1024, 8192)

    in_maps = []
    for core in range(N_CORES):
        bg, og = core // OG, core % OG
        in_maps.append(
            {
                "xT": np.ascontiguousarray(
                    xt_full[:, bg * B_SH : (bg + 1) * B_SH]
                ),
                "wv": np.ascontiguousarray(
                    wv_f16[:, og * O_SH : (og + 1) * O_SH]
                ),
            }
        )
    return in_maps


def assemble_output(results, bias):
    y_full = np.empty((B_FULL, O_FULL), dtype=np.float32)
    for core in range(N_CORES):
        bg, og = core // OG, core % OG
        y_full[bg * B_SH : (bg + 1) * B_SH, og * O_SH : (og + 1) * O_SH] = (
            np.asarray(results[core]["y"], dtype=np.float32)
        )
    y_full += bias[None, :].astype(np.float32)
    return y_full


def kernel(x, cheby_coeffs):
    nc = get_nc()
    in_maps = prep_inputs(x, cheby_coeffs)
    bias = _v_basis(cheby_coeffs)[0].sum(axis=0)
    res = run_bass_kernel_spmd(nc, in_maps, list(range(N_CORES)))
    return assemble_output(res.results, bias)
